# revision 11
# baseline (speedup 1.0000x reference)
"""Trainium2 Bass kernel for GNN message-passing encoder.

Computes (reference semantics):
    node_h = relu(emb[node_tokens] @ w1 + b1)        [N, D]
    edge_h = relu(emb[edge_tokens] @ w2 + b2)        [E, D]
    msg    = node_h[src] * edge_h                    [E, D]
    out    = segment_sum(msg, dst, N)                [N, D]

Strategy (8 NeuronCores):
  * Algebraic rewrite: both MLPs act on embedding rows, so precompute
    transformed tables R1 = relu(emb@w1+b1), R2 = relu(emb@w2+b2)
    (VOCAB rows each, f16) once, then the per-edge work is two row
    gathers (R1[node_tokens[src]], R2[edge_tokens]), an elementwise
    multiply and a segment-sum.  This removes per-edge matmuls.
  * Phase A: each core computes a 1/8 row-shard of R1 and R2 (bf16
    matmuls, relu into an SBUF staging tile, ONE contiguous DMA per
    table: the resulting token->row permutation is folded into the
    gather indices on host) and two AllGathers replicate the full
    tables.  Iteration k+1's phase A is software-pipelined into
    iteration k's phase B.
  * Phase B: edges are sorted by dst and dst-range sharded across cores
    (6272 nodes = 49 blocks of 128 per core).  Blocks are processed in
    GROUPS; per group ONE dma_gather per stream (u on SWDGE queues 0-1,
    v on 2-3) fetches every edge row of the group -- large calls
    amortize the ~1us fixed SWDGE descriptor-generation cost that
    dominated the per-block-chunked version.  Per 128-node block,
    msg = u*v on DVE, and a one-hot matrix S (DVE is_equal of the
    per-edge local dst id vs an iota row) is used on the PE:
    PSUM[128, D] += S^T @ msg accumulated over the block's edge tiles
    -- a dense segment-sum with no scatter.  Block outputs accumulate
    in SBUF (f16) and leave in a single DMA; the host undoes the
    [128, blocks*D] packing and upcasts to f32.
"""

import contextlib

import numpy as np

import concourse.bacc as bacc
import concourse.bass as bass
import concourse.mybir as mybir
import concourse.tile as tile
from concourse import library_config
from concourse.bass_utils import run_bass_kernel_spmd

F32 = mybir.dt.float32
F16 = mybir.dt.float16
BF16 = mybir.dt.bfloat16
I16 = mybir.dt.int16
_noop_ctx = contextlib.nullcontext

C = 8          # cores
D = 128        # feature dim
P = 128        # partitions


class Cfg:
    def __init__(self, n_nodes, n_edges, vocab, v_pad, blocks_pc):
        self.n_nodes = n_nodes
        self.n_edges = n_edges
        self.vocab = vocab
        self.v_pad = v_pad              # multiple of 8*128
        self.blocks_pc = blocks_pc      # node blocks (128 nodes) per core
        self.npc = blocks_pc * P        # nodes per core (padded)
        assert self.npc * C >= n_nodes
        assert v_pad % (C * P) == 0
        assert v_pad <= 32768           # int16 gather indices


FULL_CFG = Cfg(n_nodes=50000, n_edges=600000, vocab=32000, v_pad=32768,
               blocks_pc=49)

# target gather-group size in edge slots (tiles of 128); groups are runs of
# consecutive block slots whose K sums to <= GROUP_TILES
GROUP_TILES = 48


def _pi(cfg, t):
    """Token -> table-row permutation induced by the contiguous shard
    write: shard c's stage tile holds token c*4096 + j*128 + p at
    partition p, free offset j*D; the row-major DRAM row is then
    c*4096 + p*32 + j."""
    sh = cfg.v_pad // C
    st = sh // P
    c = t // sh
    o = t % sh
    j = o // P
    p = o % P
    return c * sh + p * st + j


def group_bounds(cfg, K_list, group_tiles=GROUP_TILES):
    """Split the blocks_pc slots into runs with sum(K) <= group_tiles."""
    bounds = [0]
    acc = 0
    for b, k in enumerate(K_list):
        if acc + int(k) > group_tiles and b > bounds[-1]:
            bounds.append(b)
            acc = 0
        acc += int(k)
    bounds.append(len(K_list))
    return bounds


def host_prep(cfg, emb_table, w1, bias1, w2, bias2, node_tokens, edge_tokens,
              src, dst):
    """Pure index/layout prep on host -> per-core input maps + K_list."""
    emb_table = np.asarray(emb_table, np.float32)
    w1 = np.asarray(w1, np.float32)
    w2 = np.asarray(w2, np.float32)
    bias1 = np.asarray(bias1, np.float32).reshape(1, D)
    bias2 = np.asarray(bias2, np.float32).reshape(1, D)
    node_tokens = np.asarray(node_tokens).astype(np.int64)
    edge_tokens = np.asarray(edge_tokens).astype(np.int64)
    src = np.asarray(src).astype(np.int64)
    dst = np.asarray(dst).astype(np.int64)

    stok = node_tokens[src]                      # token feeding node_h per edge
    order = np.argsort(dst, kind="stable")
    dstS = dst[order]
    stokS = stok[order]
    etokS = edge_tokens[order]

    nblk = C * cfg.blocks_pc
    bounds = np.searchsorted(dstS, np.arange(nblk + 1) * P)
    cnt = np.diff(bounds)                             # [nblk] edges per block
    # deal blocks to (core, slot) in descending-count rank order so the 8
    # blocks sharing a slot have near-equal counts -- minimizes the per-slot
    # max that pads K_list
    perm = np.argsort(-cnt, kind="stable")
    assign = perm.reshape(cfg.blocks_pc, C).T         # [C, blocks_pc] global b
    host_prep.last_assign = assign
    cnt_cs = cnt[assign]                              # [C, blocks_pc]
    nmax = cnt_cs.max(axis=0)                         # per slot max count
    K_list = np.maximum(1, -(-nmax // P)).astype(int)  # tiles per block slot
    toff = np.concatenate([[0], np.cumsum(K_list)]).astype(int)
    tiles_total = int(toff[-1])
    slots = tiles_total * P

    sh = cfg.v_pad // C

    iota = np.tile(np.arange(P, dtype=np.float16)[None, :], (P, 1))
    ones = np.ones((1, D), np.float32)

    def pack16(a):
        # gather index packing: idx k lives at [k%16, k//16], replicated to
        # 128 partitions (8 gpsimd cores x 16)
        return np.ascontiguousarray(np.tile(a.reshape(-1, 16).T, (8, 1)))

    # token -> permuted table row (contiguous shard-write layout)
    toks = np.arange(cfg.v_pad, dtype=np.int64)
    pi = _pi(cfg, toks).astype(np.int16)              # [v_pad]

    # pad indices spread over the table (duplicate-heavy index patterns
    # measured ~1.7x slower on HW than spread ones)
    spread = pi[(np.arange(slots, dtype=np.int64) * 97 % cfg.vocab)]
    in_maps = []
    for c in range(C):
        u16 = spread.copy()
        v16 = spread.copy()
        col = np.full(slots, -1.0, np.float16)
        for b in range(cfg.blocks_pc):
            gb = int(assign[c, b])
            s0, s1 = bounds[gb], bounds[gb + 1]
            m = s1 - s0
            o = toff[b] * P
            u16[o:o + m] = pi[stokS[s0:s1]]
            v16[o:o + m] = pi[etokS[s0:s1]]
            col[o:o + m] = (dstS[s0:s1] - gb * P).astype(np.float16)

        # bf16 emb shard, transposed: [128 emb-dim, sh tokens]
        shard = np.zeros((sh, D), np.float32)
        lo, hi = c * sh, min((c + 1) * sh, cfg.vocab)
        if lo < cfg.vocab:
            shard[:hi - lo] = emb_table[lo:hi]
        shardT = np.ascontiguousarray(shard.T)

        in_maps.append({
            "emb_shT": _to_bf16(shardT),                    # [128, sh] bf16
            "w1": _to_bf16(w1), "w2": _to_bf16(w2),
            "b1": _to_bf16(bias1), "b2": _to_bf16(bias2),
            "ones": _to_bf16(ones), "iota": iota,
            "idx_u": pack16(u16),                           # [128, tiles*8]
            "idx_v": pack16(v16),
            "col": np.ascontiguousarray(
                col.reshape(tiles_total, P).T),             # [128, tiles]
        })
    return in_maps, K_list


def _to_bf16(a):
    import ml_dtypes
    return np.asarray(a, np.float32).astype(ml_dtypes.bfloat16)


def build_nc(cfg, K_list, repeat_all=1, no_ag=False, no_gather=False,
             no_comp=False, no_phA=False, chunk_tiles=16, group_tiles=48,
             gat_bufs=3, qrr_all=False):
    """no_ag / no_gather / no_comp build timing-ablation variants (wrong
    results)."""
    sh = cfg.v_pad // C
    st = sh // P                      # shard tiles (phase A) per table
    toff = np.concatenate([[0], np.cumsum(K_list)]).astype(int)
    tiles_total = int(toff[-1])
    gb_list = group_bounds(cfg, K_list, group_tiles)

    nc = bacc.Bacc("TRN2", target_bir_lowering=False, num_devices=C,
                   num_swdge_queues=4)

    p_embT = nc.declare_dram_parameter("emb_shT", [P, sh], BF16,
                                       isOutput=False)
    p_w1 = nc.declare_dram_parameter("w1", [D, D], BF16, isOutput=False)
    p_w2 = nc.declare_dram_parameter("w2", [D, D], BF16, isOutput=False)
    p_b1 = nc.declare_dram_parameter("b1", [1, D], BF16, isOutput=False)
    p_b2 = nc.declare_dram_parameter("b2", [1, D], BF16, isOutput=False)
    p_ones = nc.declare_dram_parameter("ones", [1, D], BF16, isOutput=False)
    p_iota = nc.declare_dram_parameter("iota", [P, P], F16, isOutput=False)
    p_idxu = nc.declare_dram_parameter("idx_u", [P, tiles_total * 8], I16,
                                       isOutput=False)
    p_idxv = nc.declare_dram_parameter("idx_v", [P, tiles_total * 8], I16,
                                       isOutput=False)
    p_col = nc.declare_dram_parameter("col", [P, tiles_total], F16,
                                      isOutput=False)
    p_out = nc.declare_dram_parameter("out", [P, cfg.blocks_pc * D], F16,
                                      isOutput=True)

    with tile.TileContext(nc) as tc:
        with (
            tc.tile_pool(name="dram", bufs=2, space="DRAM") as dramp,
            tc.tile_pool(name="cst", bufs=1) as cst,
            tc.tile_pool(name="stg", bufs=2) as stg,
            tc.tile_pool(name="psA", bufs=2, space="PSUM") as psA,
            tc.tile_pool(name="gat", bufs=gat_bufs) as gat,
            tc.tile_pool(name="sm", bufs=6) as smp,
            tc.tile_pool(name="psB", bufs=4, space="PSUM") as psB,
            tc.tile_pool(name="fl", bufs=2) as flp,
        ):
            w1_sb = cst.tile([D, D], BF16)
            nc.sync.dma_start(w1_sb[:], p_w1[:])
            w2_sb = cst.tile([D, D], BF16)
            nc.sync.dma_start(w2_sb[:], p_w2[:])
            b1_sb = cst.tile([1, D], BF16)
            nc.sync.dma_start(b1_sb[:], p_b1[:])
            b2_sb = cst.tile([1, D], BF16)
            nc.sync.dma_start(b2_sb[:], p_b2[:])
            ones_sb = cst.tile([1, D], BF16)
            nc.sync.dma_start(ones_sb[:], p_ones[:])
            iota_sb = cst.tile([P, P], F16)
            nc.sync.dma_start(iota_sb[:], p_iota[:])
            embT_sb = cst.tile([P, sh], BF16)
            nc.sync.dma_start(embT_sb[:], p_embT[:])
            idxu_sb = cst.tile([P, tiles_total * 8], I16)
            nc.sync.dma_start(idxu_sb[:], p_idxu[:])
            idxv_sb = cst.tile([P, tiles_total * 8], I16)
            nc.sync.dma_start(idxv_sb[:], p_idxv[:])
            col_sb = cst.tile([P, tiles_total], F16)
            nc.sync.dma_start(col_sb[:], p_col[:])

            ni_regs = {}

            def _reg(ni):
                if ni not in ni_regs:
                    ni_regs[ni] = nc.gpsimd.to_reg(ni)
                return ni_regs[ni]

            iota3 = iota_sb[:].rearrange("p (k j) -> p k j", k=1)

            _emit(nc, cfg, K_list, toff, gb_list, sh, st, _reg, iota3,
                  dramp, stg, psA, gat, smp, psB, flp, w1_sb, w2_sb,
                  b1_sb, b2_sb, ones_sb, col_sb, embT_sb, idxu_sb,
                  idxv_sb, p_out, repeat_all, no_ag, no_gather, no_comp,
                  no_phA, chunk_tiles, qrr_all)

    nc.compile()
    return nc


def _emit(nc, cfg, K_list, toff, gb_list, sh, st, _reg, iota3, dramp, stg,
          psA, gat, smp, psB, flp, w1_sb, w2_sb, b1_sb, b2_sb, ones_sb,
          col_sb, embT_sb, idxu_sb, idxv_sb, p_out, repeat_all, no_ag,
          no_gather, no_comp, no_phA, chunk_tiles, qrr_all):
    NA = 3             # phase A items injected per phase B block

    def new_tables():
        shard1 = dramp.tile([sh, D], F16, tag="shard1")
        shard2 = dramp.tile([sh, D], F16, tag="shard2")
        full1 = dramp.tile([cfg.v_pad, D], F16, addr_space="Shared",
                           tag="full1")
        full2 = dramp.tile([cfg.v_pad, D], F16, addr_space="Shared",
                           tag="full2")
        return full1, full2, shard1, shard2

    def emit_ag(shard, full):
        if no_ag:
            return
        nc.gpsimd.collective_compute(
            "AllGather",
            mybir.AluOpType.bypass,
            replica_groups=[list(range(C))],
            ins=[shard.opt()],
            outs=[full.opt()],
        )

    def a_items(tabs):
        """Closure list: 2*(st matmul/relu tiles + 1 flush (DMA+AG))."""
        full1, full2, shard1, shard2 = tabs
        items = []
        for w_sb, b_sb, shard, full in ((w1_sb, b1_sb, shard1, full1),
                                        (w2_sb, b2_sb, shard2, full2)):
            stage = stg.tile([P, sh], F16, tag="stage")

            def tile_fn(j, w_sb=w_sb, b_sb=b_sb, stage=stage):
                ps = psA.tile([P, D], F32)
                emb_j = embT_sb[:, j * P:(j + 1) * P]
                nc.tensor.matmul(ps[:], lhsT=emb_j, rhs=w_sb[:],
                                 start=True, stop=False)
                nc.tensor.matmul(ps[:], lhsT=ones_sb[:], rhs=b_sb[:],
                                 start=False, stop=True)
                nc.scalar.activation(stage[:, j * D:(j + 1) * D], ps[:],
                                     mybir.ActivationFunctionType.Relu)

            for j in range(st):
                items.append(lambda j=j, f=tile_fn: f(j))

            def flush(stage=stage, shard=shard, full=full):
                dst = shard[:].rearrange("(p j) d -> p (j d)", p=P)
                nc.sync.dma_start(dst, stage[:])
                emit_ag(shard, full)

            items.append(flush)
        return items

    # iteration 0's tables fully up front
    tabs = new_tables()
    for it_fn in a_items(tabs):
        it_fn()

    qu = [0]
    qv = [0]
    ngroups = len(gb_list) - 1
    for it in range(repeat_all):
        full1, full2 = tabs[0], tabs[1]
        ntabs = None if no_phA else (
            new_tables() if it + 1 < repeat_all else None)
        pend = a_items(ntabs) if ntabs is not None else []
        na_done = 0
        out_sb = flp.tile([P, cfg.blocks_pc * D], F16, tag="out")
        if no_comp:
            nc.vector.memset(out_sb[:], 0.0)
        bi = 0
        for g in range(ngroups):
            s0, s1 = gb_list[g], gb_list[g + 1]
            t0, t1 = int(toff[s0]), int(toff[s1])
            kg = t1 - t0
            ni = kg * P
            ub = vb = None
            if not no_gather:
                ub = gat.tile([P, kg * D], F16, tag="ub")
                vb = gat.tile([P, kg * D], F16, tag="vb")
                CH = chunk_tiles     # tiles per gather call
                for (buf, full, idx_sb, qrr, qbase) in (
                        (ub, full1, idxu_sb, qu, 0),
                        (vb, full2, idxv_sb, qv, 2)):
                    for c0 in range(0, kg, CH):
                        cn = min(CH, kg - c0)
                        if qrr_all:
                            q = qu[0] % 4
                            qu[0] += 1
                        else:
                            q = qbase + qrr[0] % 2
                            qrr[0] += 1
                        nc.gpsimd.dma_gather(
                            out_ap=buf[:, c0 * D:(c0 + cn) * D].rearrange(
                                "p (k d) -> p k d", d=D),
                            in_ap=full[:],
                            idxs_ap=idx_sb[:, (t0 + c0) * 8:
                                           (t0 + c0 + cn) * 8],
                            num_idxs=cn * P,
                            num_idxs_reg=_reg(cn * P),
                            elem_size=D,
                            elem_step=D,
                            single_packet=False,
                            queue_num=q,
                        )
            for b in range(s0, s1):
                if no_comp:
                    if ntabs is not None and bi >= 1 and na_done < len(pend):
                        for it_fn in pend[na_done:na_done + NA]:
                            it_fn()
                        na_done += NA
                    bi += 1
                    continue
                K = int(K_list[b])
                lt = int(toff[b]) - t0          # local tile offset in group
                g0 = int(toff[b])
                Sw = smp.tile([P, K * P], F16, tag="S")
                nc.vector.tensor_tensor(
                    out=Sw[:].rearrange("p (k j) -> p k j", j=P),
                    in0=col_sb[:, g0:g0 + K].to_broadcast([P, K, P]),
                    in1=iota3.to_broadcast([P, K, P]),
                    op=mybir.AluOpType.is_equal,
                )
                if no_gather:
                    mw = Sw         # timing ablation: garbage msg values
                else:
                    mw = smp.tile([P, K * D], F16, tag="m")
                    nc.vector.tensor_tensor(
                        out=mw[:], in0=ub[:, lt * D:(lt + K) * D],
                        in1=vb[:, lt * D:(lt + K) * D],
                        op=mybir.AluOpType.mult,
                    )
                ps = psB.tile([P, D], F32)
                for t in range(K):
                    nc.tensor.matmul(ps[:], lhsT=Sw[:, t * P:(t + 1) * P],
                                     rhs=mw[:, t * D:(t + 1) * D],
                                     start=(t == 0), stop=(t == K - 1))
                nc.scalar.activation(out_sb[:, b * D:(b + 1) * D], ps[:],
                                     mybir.ActivationFunctionType.Copy)
                if ntabs is not None and bi >= 1 and na_done < len(pend):
                    for it_fn in pend[na_done:na_done + NA]:
                        it_fn()
                    na_done += NA
                bi += 1
        # any phase A stragglers (small blocks_pc or large NA)
        if ntabs is not None and na_done < len(pend):
            for it_fn in pend[na_done:]:
                it_fn()
        nc.sync.dma_start(p_out[:], out_sb[:])
        if not no_phA:
            tabs = ntabs


_nc_cache = {}


def kernel(emb_table, w1, bias1, w2, bias2, node_tokens, edge_tokens, src,
           dst):
    cfg = FULL_CFG
    in_maps, K_list = host_prep(cfg, emb_table, w1, bias1, w2, bias2,
                                node_tokens, edge_tokens, src, dst)
    key = tuple(int(k) for k in K_list)
    if key not in _nc_cache:
        _nc_cache[key] = build_nc(cfg, K_list)
    assign = host_prep.last_assign
    res = run_bass_kernel_spmd(_nc_cache[key], in_maps,
                               core_ids=list(range(C)))
    # out is packed [128, blocks*D] f16 per core; slot b of core c holds
    # global dst block assign[c, b] (nodes gb*128 .. gb*128+128)
    out = np.zeros((C * cfg.npc, D), np.float32)
    for c in range(C):
        o = res.results[c]["out"].astype(np.float32).reshape(
            P, cfg.blocks_pc, D)
        for b in range(cfg.blocks_pc):
            gb = int(assign[c, b])
            out[gb * P:(gb + 1) * P] = o[:, b, :]
    return np.ascontiguousarray(out[:cfg.n_nodes]).astype(np.float32)
